# revision 7
# baseline (speedup 1.0000x reference)
"""Causal self-attention (B=2, T=2048, C=1024, H=16, D=64) on 8 Trainium2 cores.

Sharding: head-parallel. Core c owns heads {2c, 2c+1} for both batches:
  - qkv projection with its 128-column slice of W_qkv (tensor parallel),
  - attention for its 4 (batch, head) pairs,
  - AllGather of per-core attention outputs Y^T (128 rows each -> 1024),
  - output projection against a 128-column slice of W_out (tensor parallel).
Host assembles the 8 column slices into the full output.

On-chip layout is fully transposed (feature dims on partitions, tokens on the
free axis) so every matmul streams 512-wide moving operands:
  scoresT[s,t] = kT.T @ qT  (K=64, both heads row-tiled into the PE array)
  expT = Exp(scoresT/8)     (ACT, straight from PSUM; causal mask added to
                             PSUM as a 0/-1e9 tile before the exp)
  av    = [v | 1].T @ expT  (K=128; ones column makes row 64 the softmax
                             denominator -- no separate reduction pass)
  YT    = av[0:64] * bcast(1/av[64])  (reciprocal + K=1 ones matmul bcast)
"""
import numpy as np

import concourse.bass as bass
import concourse.mybir as mybir
import concourse.tile as tile
from concourse import bacc
from concourse.masks import make_identity
from concourse.bass_utils import run_bass_kernel_spmd

AF = mybir.ActivationFunctionType
FP32 = mybir.dt.float32

B, T, C, H, D = 2, 2048, 1024, 16, 64
N_CORES = 8
BT = B * T                      # 4096 tokens
NT = BT // 512                  # 8 token tiles of 512
NK = C // 128                   # 8 contraction tiles
HPC = H // N_CORES              # 2 heads per core
CW = HPC * D                    # 128 columns of Y / W_out per core
JT = T // 512                   # 4 token tiles per batch
ST = T // 128                   # 16 s-tiles per batch
VBLK = 2 * (D + 1)              # 130: [v_h0(64) | 1 | v_h1(64) | 1]

# matmul input dtype: "f32r" (tf32-like, full PE rate), "fp32", "bf16"
MM_DTYPE = "f32r"

_nc_cache = {}


def _build_nc(mm_dtype: str):
    DT = {"f32r": mybir.dt.float32r, "fp32": mybir.dt.float32,
          "bf16": mybir.dt.bfloat16}[mm_dtype]

    nc = bacc.Bacc("TRN2", target_bir_lowering=False, debug=False,
                   num_devices=N_CORES)

    xT = nc.dram_tensor("xT", [C, BT], DT, kind="ExternalInput").ap()
    wqkv = nc.dram_tensor("wqkv", [C, 3 * CW], DT, kind="ExternalInput").ap()
    bqkv = nc.dram_tensor("bqkv", [CW, 3], FP32, kind="ExternalInput").ap()
    wout = nc.dram_tensor("wout", [C, CW], DT, kind="ExternalInput").ap()
    bout = nc.dram_tensor("bout", [CW, 1], FP32, kind="ExternalInput").ap()
    cones = nc.dram_tensor("cones", [CW, 2 * ST + 2], DT,
                           kind="ExternalInput").ap()
    outT = nc.dram_tensor("outT", [CW, BT], FP32, kind="ExternalOutput").ap()

    wqkv3 = wqkv.rearrange("(ko p) m -> p ko m", p=128)   # [128, 8, 384]
    wout3 = wout.rearrange("(ko p) m -> p ko m", p=128)   # [128, 8, 128]

    with tile.TileContext(nc) as tc:
        with tc.tile_pool(name="persist", bufs=1) as pp, \
             tc.tile_pool(name="dram", bufs=1, space="DRAM") as dram:
            qTs = pp.tile([128, BT], DT)
            kTs = pp.tile([128, BT], DT)
            vs = pp.tile([128, 2 * ST, VBLK], DT)
            YT = pp.tile([128, BT], DT)
            wq_s = pp.tile([128, NK, 3 * CW], DT)
            wo_s = pp.tile([128, NK, CW], DT)
            bq_s = pp.tile([CW, 3], FP32)
            bo_s = pp.tile([CW, 1], FP32)
            ones_s = pp.tile([1, 128], DT)
            ident = pp.tile([128, 128], FP32)
            nmask = pp.tile([128, 4, 512], FP32)

            nc.sync.dma_start(wq_s[:], wqkv3[:])
            nc.sync.dma_start(wo_s[:], wout3[:])
            nc.sync.dma_start(bq_s[:], bqkv[:])
            nc.sync.dma_start(bo_s[:], bout[:])
            nc.sync.dma_start(ones_s[:],
                              cones.rearrange("p s -> s p")[0:1, :])
            # ones columns of vs (positions 64 and 129 of each 130-block)
            nc.sync.dma_start(vs[:, :, D:D + 1],
                              cones[:, 0:2 * ST].unsqueeze(2))
            nc.sync.dma_start(vs[:, :, 2 * D + 1:2 * D + 2],
                              cones[:, 0:2 * ST].unsqueeze(2))
            make_identity(nc, ident[:])
            # nmask[dm][p,f] = 0 where f - p - 128*dm >= 0 else -1e9
            for dm in range(4):
                nc.gpsimd.memset(nmask[:, dm, :], 0.0)
                nc.gpsimd.affine_select(
                    out=nmask[:, dm, :], in_=nmask[:, dm, :],
                    compare_op=mybir.AluOpType.is_ge,
                    fill=-1e9, base=-128 * dm, channel_multiplier=-1,
                    pattern=[[1, 512]],
                )

            ag_in = dram.tile([128, BT], DT)
            ag_out = dram.tile([128 * N_CORES, BT], DT)

            # ---------------- phase 1: q/k/v projections -----------------
            with tc.tile_pool(name="ph1_sb", bufs=3) as sb1, \
                 tc.tile_pool(name="ph1_ps", bufs=2, space="PSUM") as ps1:
                for n in range(NT):
                    ncol = slice(n * 512, (n + 1) * 512)
                    pq = ps1.tile([128, 512], FP32, tag="pq")
                    pk = ps1.tile([128, 512], FP32, tag="pk")
                    pv = ps1.tile([128, 512], FP32, tag="pv")
                    for k in range(NK):
                        xt = sb1.tile([128, 512], DT, tag="xt")
                        nc.sync.dma_start(xt[:], xT[k * 128:(k + 1) * 128, ncol])
                        st, sp = (k == 0), (k == NK - 1)
                        nc.tensor.matmul(pq[:], wq_s[:, k, 0:128], xt[:],
                                         start=st, stop=sp)
                        nc.tensor.matmul(pk[:], wq_s[:, k, 128:256], xt[:],
                                         start=st, stop=sp)
                        nc.tensor.matmul(pv[:], wq_s[:, k, 256:384], xt[:],
                                         start=st, stop=sp)
                    nc.scalar.activation(qTs[:, ncol], pq[:], AF.Identity,
                                         bias=bq_s[:, 0:1])
                    nc.scalar.activation(kTs[:, ncol], pk[:], AF.Identity,
                                         bias=bq_s[:, 1:2])
                    vt = sb1.tile([128, 512], FP32, tag="vt")
                    nc.scalar.activation(vt[:], pv[:], AF.Identity,
                                         bias=bq_s[:, 2:3])
                    for t4 in range(4):
                        gi = n * 4 + t4
                        ptr = ps1.tile([128, 128], FP32, tag="ptr")
                        nc.tensor.transpose(ptr[:], vt[:, t4 * 128:(t4 + 1) * 128],
                                            ident[:])
                        nc.scalar.activation(vs[:, gi, 0:D], ptr[:, 0:D],
                                             AF.Copy)
                        nc.scalar.activation(vs[:, gi, D + 1:2 * D + 1],
                                             ptr[:, D:2 * D], AF.Copy)

            # ---------------- phase 2: attention -------------------------
            with tc.tile_pool(name="ph2_e", bufs=4) as sbe, \
                 tc.tile_pool(name="ph2_s", bufs=4) as sbs, \
                 tc.tile_pool(name="ps_sc", bufs=2, space="PSUM") as ps_sc, \
                 tc.tile_pool(name="ps_av", bufs=1, space="PSUM") as ps_av, \
                 tc.tile_pool(name="ps_bc", bufs=2, space="PSUM") as ps_bc:
                for b in range(B):
                    for j in range(JT):
                        col = slice(b * T + j * 512, b * T + (j + 1) * 512)
                        n_i = 4 * j + 4
                        av = [ps_av.tile([128, 512], FP32, tag=f"av{h}",
                                          name=f"av{h}_{b}_{j}")
                              for h in range(2)]
                        for i in range(n_i):
                            krange = slice(b * T + i * 128, b * T + (i + 1) * 128)
                            gi = b * ST + i
                            st, sp = (i == 0), (i == n_i - 1)
                            for h in range(2):
                                hp = slice(h * D, (h + 1) * D)
                                ps = ps_sc.tile([128, 512], FP32, tag=f"s{h}")
                                nc.tensor.matmul(
                                    ps[:], kTs[hp, krange], qTs[hp, col],
                                    start=True, stop=True,
                                    tile_position=(h * D, 0))
                                if i >= 4 * j:
                                    nc.vector.tensor_add(
                                        ps[:], ps[:], nmask[:, i - 4 * j, :])
                                e = sbe.tile([128, 512], DT, tag=f"e{h}")
                                nc.scalar.activation(e[:], ps[:], AF.Exp,
                                                     scale=0.125)
                                nc.tensor.matmul(
                                    av[h][0:D + 1, :],
                                    vs[:, gi, h * (D + 1):(h + 1) * (D + 1)],
                                    e[:], start=st, stop=sp)
                        for h in range(2):
                            recip = sbs.tile([1, 512], FP32, tag="recip")
                            nc.vector.reciprocal(recip[:], av[h][D:D + 1, :])
                            rr = sbs.tile([1, 512], DT, tag="rr")
                            nc.scalar.activation(rr[:], recip[:], AF.Copy)
                            pbc = ps_bc.tile([128, 512], FP32, tag="bc")
                            nc.tensor.matmul(pbc[:], ones_s[:], rr[:],
                                             start=True, stop=True)
                            avs = sbs.tile([D, 512], FP32, tag="avs")
                            nc.vector.tensor_copy(avs[:], av[h][0:D, :])
                            nc.vector.tensor_mul(
                                YT[h * D:(h + 1) * D, col],
                                avs[:], pbc[0:D, :])

            # ---------------- phase 3: AllGather + out projection ---------
            nc.sync.dma_start(ag_in[:], YT[:])
            nc.gpsimd.collective_compute(
                "AllGather", mybir.AluOpType.bypass,
                replica_groups=[list(range(N_CORES))],
                ins=[ag_in.opt()], outs=[ag_out.opt()],
            )
            with tc.tile_pool(name="ph3_sb", bufs=3) as sb3, \
                 tc.tile_pool(name="ph3_ps", bufs=2, space="PSUM") as ps3:
                for n in range(NT):
                    ncol = slice(n * 512, (n + 1) * 512)
                    p3 = ps3.tile([128, 512], FP32, tag="p3")
                    for k in range(NK):
                        yt = sb3.tile([128, 512], DT, tag="yt")
                        nc.sync.dma_start(
                            yt[:], ag_out[k * 128:(k + 1) * 128, ncol])
                        nc.tensor.matmul(p3[:], wo_s[:, k, :], yt[:],
                                         start=(k == 0), stop=(k == NK - 1))
                    ot = sb3.tile([128, 512], FP32, tag="ot")
                    nc.scalar.activation(ot[:], p3[:], AF.Identity,
                                         bias=bo_s[:, 0:1])
                    nc.sync.dma_start(outT[:, ncol], ot[:])

    nc.compile()
    return nc


def _get_nc():
    if MM_DTYPE not in _nc_cache:
        _nc_cache[MM_DTYPE] = _build_nc(MM_DTYPE)
    return _nc_cache[MM_DTYPE]


def _np_dt(mm_dtype: str):
    if mm_dtype == "bf16":
        import ml_dtypes
        return ml_dtypes.bfloat16
    return np.float32


def make_in_maps(x, W_qkv, b_qkv, W_out, b_out):
    ndt = _np_dt(MM_DTYPE)
    xT = np.ascontiguousarray(
        x.reshape(BT, C).T).astype(ndt)               # [C, BT]
    cones = np.ones((CW, 2 * ST + 2), dtype=ndt)
    in_maps = []
    for c in range(N_CORES):
        h0 = c * CW                                    # first q/k/v column
        wq = W_qkv[:, h0:h0 + CW]
        wk = W_qkv[:, C + h0:C + h0 + CW]
        wv = W_qkv[:, 2 * C + h0:2 * C + h0 + CW]
        wqkv = np.ascontiguousarray(
            np.concatenate([wq, wk, wv], axis=1)).astype(ndt)
        bq = np.stack([b_qkv[h0:h0 + CW], b_qkv[C + h0:C + h0 + CW],
                       b_qkv[2 * C + h0:2 * C + h0 + CW]],
                      axis=1).astype(np.float32)       # [128, 3]
        wo = np.ascontiguousarray(W_out[:, h0:h0 + CW]).astype(ndt)
        bo = b_out[h0:h0 + CW].reshape(CW, 1).astype(np.float32)
        in_maps.append({
            "xT": xT, "wqkv": wqkv, "bqkv": bq,
            "wout": wo, "bout": bo, "cones": cones,
        })
    return in_maps


def run(in_maps, trace=False):
    nc = _get_nc()
    return run_bass_kernel_spmd(nc, in_maps, core_ids=list(range(N_CORES)),
                                trace=trace)


def kernel(x, W_qkv, b_qkv, W_out, b_out):
    x = np.asarray(x, dtype=np.float32)
    W_qkv = np.asarray(W_qkv, dtype=np.float32)
    b_qkv = np.asarray(b_qkv, dtype=np.float32)
    W_out = np.asarray(W_out, dtype=np.float32)
    b_out = np.asarray(b_out, dtype=np.float32)

    in_maps = make_in_maps(x, W_qkv, b_qkv, W_out, b_out)
    res = run(in_maps, trace=False)
    full_outT = np.concatenate(
        [res.results[c]["outT"] for c in range(N_CORES)], axis=0)  # [C, BT]
    return np.ascontiguousarray(full_outT.T).reshape(B, T, C)


# revision 8
# speedup vs baseline: 1.0460x; 1.0460x over previous
"""Causal self-attention (B=2, T=2048, C=1024, H=16, D=64) on 8 Trainium2 cores.

Sharding: head-parallel. Core c owns heads {2c, 2c+1} for both batches:
  - qkv projection with its 128-column slice of W_qkv (tensor parallel),
  - attention for its 4 (batch, head) pairs,
  - per-batch AllGather of per-core attention outputs Y^T (128 rows -> 1024),
  - output projection against a 128-column slice of W_out (tensor parallel).
Host assembles the 8 column slices into the full output.

On-chip layout is fully transposed (feature dims on partitions, tokens on the
free axis) so every matmul streams 512-wide moving operands:
  scoresT[s,t] = kT.T @ qT  (K=64, both heads row-tiled into the PE array)
  expT = Exp(scoresT/8)     (ACT, straight from PSUM; causal mask added to
                             PSUM as a 0/-1e9 tile before the exp)
  av    = [v | 1].T @ expT  (K=128; ones column makes row 64 the softmax
                             denominator -- no separate reduction pass)
  YT    = av[0:64] * bcast(1/av[64])  (reciprocal + K=1 ones matmul bcast)

All DRAM traffic is tile-major ([... ,128 ,512] contiguous blocks) so each
DMA is one 128-512KB descriptor run instead of 128 strided 1-2KB rows.
The AllGather is split per batch: AG(b=0) overlaps attention for b=1, and
the b=0 output projection overlaps AG(b=1).
"""
import numpy as np

import concourse.bass as bass
import concourse.mybir as mybir
import concourse.tile as tile
from concourse import bacc
from concourse.masks import make_identity
from concourse.bass_utils import run_bass_kernel_spmd

AF = mybir.ActivationFunctionType
FP32 = mybir.dt.float32

B, T, C, H, D = 2, 2048, 1024, 16, 64
N_CORES = 8
BT = B * T                      # 4096 tokens
NT = BT // 512                  # 8 token tiles of 512
NK = C // 128                   # 8 contraction tiles
HPC = H // N_CORES              # 2 heads per core
CW = HPC * D                    # 128 columns of Y / W_out per core
JT = T // 512                   # 4 token tiles per batch
ST = T // 128                   # 16 s-tiles per batch
VBLK = 2 * (D + 1)              # 130: [v_h0(64) | 1 | v_h1(64) | 1]

# matmul input dtype: "f32r" (tf32-like), "fp32", "bf16"
MM_DTYPE = "f32r"

_nc_cache = {}


def _build_nc(mm_dtype: str):
    DT = {"f32r": mybir.dt.float32r, "fp32": mybir.dt.float32,
          "bf16": mybir.dt.bfloat16}[mm_dtype]

    nc = bacc.Bacc("TRN2", target_bir_lowering=False, debug=False,
                   num_devices=N_CORES)

    # tile-major inputs: every [128, 512] block is contiguous in DRAM
    xT = nc.dram_tensor("xT", [NK, NT, 128, 512], DT,
                        kind="ExternalInput").ap()
    wqkv = nc.dram_tensor("wqkv", [C, 3 * CW], DT, kind="ExternalInput").ap()
    bqkv = nc.dram_tensor("bqkv", [CW, 3], FP32, kind="ExternalInput").ap()
    wout = nc.dram_tensor("wout", [C, CW], DT, kind="ExternalInput").ap()
    bout = nc.dram_tensor("bout", [CW, 1], FP32, kind="ExternalInput").ap()
    cones = nc.dram_tensor("cones", [CW, 2 * ST + 2], DT,
                           kind="ExternalInput").ap()
    outT = nc.dram_tensor("outT", [NT, 128, 512], FP32,
                          kind="ExternalOutput").ap()

    wqkv3 = wqkv.rearrange("(ko p) m -> p ko m", p=128)   # [128, 8, 384]
    wout3 = wout.rearrange("(ko p) m -> p ko m", p=128)   # [128, 8, 128]

    with tile.TileContext(nc) as tc:
        with tc.tile_pool(name="persist", bufs=1) as pp, \
             tc.tile_pool(name="dram", bufs=1, space="DRAM") as dram:
            qTs = pp.tile([128, BT], DT)
            kTs = pp.tile([128, BT], DT)
            vs = pp.tile([128, 2 * ST, VBLK], DT)
            YT = pp.tile([128, BT], DT)
            wq_s = pp.tile([128, NK, 3 * CW], DT)
            wo_s = pp.tile([128, NK, CW], DT)
            bq_s = pp.tile([CW, 3], FP32)
            bo_s = pp.tile([CW, 1], FP32)
            ones_s = pp.tile([1, 128], DT)
            ident = pp.tile([128, 128], FP32)
            nmask = pp.tile([128, 4, 512], FP32)

            nc.sync.dma_start(wq_s[:], wqkv3[:])
            nc.sync.dma_start(wo_s[:], wout3[:])
            nc.sync.dma_start(bq_s[:], bqkv[:])
            nc.sync.dma_start(bo_s[:], bout[:])
            nc.sync.dma_start(ones_s[:],
                              cones.rearrange("p s -> s p")[0:1, :])
            # ones columns of vs (positions 64 and 129 of each 130-block)
            nc.sync.dma_start(vs[:, :, D:D + 1],
                              cones[:, 0:2 * ST].unsqueeze(2))
            nc.sync.dma_start(vs[:, :, 2 * D + 1:2 * D + 2],
                              cones[:, 0:2 * ST].unsqueeze(2))
            make_identity(nc, ident[:])
            # nmask[dm][p,f] = 0 where f - p - 128*dm >= 0 else -1e9
            for dm in range(4):
                nc.gpsimd.memset(nmask[:, dm, :], 0.0)
                nc.gpsimd.affine_select(
                    out=nmask[:, dm, :], in_=nmask[:, dm, :],
                    compare_op=mybir.AluOpType.is_ge,
                    fill=-1e9, base=-128 * dm, channel_multiplier=-1,
                    pattern=[[1, 512]],
                )

            # per-batch AllGather buffers, tile-major
            ag_in = [dram.tile([JT, 128, 512], DT, name=f"ag_in{b}")
                     for b in range(B)]
            ag_out = [dram.tile([N_CORES * JT, 128, 512], DT,
                                name=f"ag_out{b}") for b in range(B)]

            # ---------------- phase 1: q/k/v projections -----------------
            with tc.tile_pool(name="ph1_sb", bufs=8) as sb1, \
                 tc.tile_pool(name="ph1_vt", bufs=3) as sbv, \
                 tc.tile_pool(name="ph1_ps", bufs=2, space="PSUM") as ps1:
                for n in range(NT):
                    ncol = slice(n * 512, (n + 1) * 512)
                    pq = ps1.tile([128, 512], FP32, tag="pq")
                    pk = ps1.tile([128, 512], FP32, tag="pk")
                    pv = ps1.tile([128, 512], FP32, tag="pv")
                    for k in range(NK):
                        xt = sb1.tile([128, 512], DT, tag="xt")
                        nc.sync.dma_start(xt[:], xT[k, n])
                        st, sp = (k == 0), (k == NK - 1)
                        nc.tensor.matmul(pq[:], wq_s[:, k, 0:128], xt[:],
                                         start=st, stop=sp)
                        nc.tensor.matmul(pk[:], wq_s[:, k, 128:256], xt[:],
                                         start=st, stop=sp)
                        nc.tensor.matmul(pv[:], wq_s[:, k, 256:384], xt[:],
                                         start=st, stop=sp)
                    nc.scalar.activation(qTs[:, ncol], pq[:], AF.Identity,
                                         bias=bq_s[:, 0:1])
                    nc.scalar.activation(kTs[:, ncol], pk[:], AF.Identity,
                                         bias=bq_s[:, 1:2])
                    vt = sbv.tile([128, 512], FP32, tag="vt")
                    nc.scalar.activation(vt[:], pv[:], AF.Identity,
                                         bias=bq_s[:, 2:3])
                    for t4 in range(4):
                        gi = n * 4 + t4
                        ptr = ps1.tile([128, 128], FP32, tag="ptr")
                        nc.tensor.transpose(ptr[:], vt[:, t4 * 128:(t4 + 1) * 128],
                                            ident[:])
                        nc.scalar.activation(vs[:, gi, 0:D], ptr[:, 0:D],
                                             AF.Copy)
                        nc.scalar.activation(vs[:, gi, D + 1:2 * D + 1],
                                             ptr[:, D:2 * D], AF.Copy)

            # -------- phase 2 (attention) + split AG + phase 3 ------------
            with tc.tile_pool(name="ph2_e", bufs=6) as sbe, \
                 tc.tile_pool(name="ph2_s", bufs=4) as sbs, \
                 tc.tile_pool(name="ph3_sb", bufs=6) as sb3, \
                 tc.tile_pool(name="ps_sc", bufs=2, space="PSUM") as ps_sc, \
                 tc.tile_pool(name="ps_av", bufs=1, space="PSUM") as ps_av, \
                 tc.tile_pool(name="ps_bc", bufs=1, space="PSUM") as ps_bc, \
                 tc.tile_pool(name="ph3_ps", bufs=1, space="PSUM") as ps3:

                def attention_batch(b):
                    for j in range(JT):
                        col = slice(b * T + j * 512, b * T + (j + 1) * 512)
                        n_i = 4 * j + 4
                        av = [ps_av.tile([128, 512], FP32, tag=f"av{h}",
                                         name=f"av{h}_{b}_{j}")
                              for h in range(2)]
                        for i in range(n_i):
                            krange = slice(b * T + i * 128,
                                           b * T + (i + 1) * 128)
                            gi = b * ST + i
                            st, sp = (i == 0), (i == n_i - 1)
                            for h in range(2):
                                hp = slice(h * D, (h + 1) * D)
                                ps = ps_sc.tile([128, 512], FP32, tag=f"s{h}")
                                nc.tensor.matmul(
                                    ps[:], kTs[hp, krange], qTs[hp, col],
                                    start=True, stop=True,
                                    tile_position=(h * D, 0))
                                if i >= 4 * j:
                                    nc.vector.tensor_add(
                                        ps[:], ps[:], nmask[:, i - 4 * j, :])
                                e = sbe.tile([128, 512], DT, tag=f"e{h}")
                                nc.scalar.activation(e[:], ps[:], AF.Exp,
                                                     scale=0.125)
                                nc.tensor.matmul(
                                    av[h][0:D + 1, :],
                                    vs[:, gi, h * (D + 1):(h + 1) * (D + 1)],
                                    e[:], start=st, stop=sp)
                        for h in range(2):
                            recip = sbs.tile([1, 512], FP32, tag="recip")
                            nc.vector.reciprocal(recip[:], av[h][D:D + 1, :])
                            rr = sbs.tile([1, 512], DT, tag="rr")
                            nc.scalar.activation(rr[:], recip[:], AF.Copy)
                            pbc = ps_bc.tile([128, 512], FP32, tag="bc")
                            nc.tensor.matmul(pbc[:], ones_s[:], rr[:],
                                             start=True, stop=True)
                            avs = sbs.tile([D, 512], FP32, tag="avs")
                            nc.vector.tensor_copy(avs[:], av[h][0:D, :])
                            nc.vector.tensor_mul(
                                YT[h * D:(h + 1) * D, col],
                                avs[:], pbc[0:D, :])
                        # ship this j-tile of YT to the AG input buffer
                        nc.sync.dma_start(ag_in[b][j], YT[:, col])

                def gather_batch(b):
                    nc.gpsimd.collective_compute(
                        "AllGather", mybir.AluOpType.bypass,
                        replica_groups=[list(range(N_CORES))],
                        ins=[ag_in[b].opt()], outs=[ag_out[b].opt()],
                    )

                def project_batch(b):
                    for j in range(JT):
                        n = b * JT + j
                        p3 = ps3.tile([128, 512], FP32, tag="p3",
                                      name=f"p3_{b}_{j}")
                        for r in range(NK):
                            yt = sb3.tile([128, 512], DT, tag="yt")
                            nc.sync.dma_start(yt[:], ag_out[b][r * JT + j])
                            nc.tensor.matmul(p3[:], wo_s[:, r, :], yt[:],
                                             start=(r == 0), stop=(r == NK - 1))
                        ot = sb3.tile([128, 512], FP32, tag="ot")
                        nc.scalar.activation(ot[:], p3[:], AF.Identity,
                                             bias=bo_s[:, 0:1])
                        nc.sync.dma_start(outT[n], ot[:])

                attention_batch(0)
                gather_batch(0)
                attention_batch(1)
                gather_batch(1)
                project_batch(0)
                project_batch(1)

    nc.compile()
    return nc


def _get_nc():
    if MM_DTYPE not in _nc_cache:
        _nc_cache[MM_DTYPE] = _build_nc(MM_DTYPE)
    return _nc_cache[MM_DTYPE]


def _np_dt(mm_dtype: str):
    if mm_dtype == "bf16":
        import ml_dtypes
        return ml_dtypes.bfloat16
    return np.float32


def make_in_maps(x, W_qkv, b_qkv, W_out, b_out):
    ndt = _np_dt(MM_DTYPE)
    xT = x.reshape(BT, C).T                                # [C, BT]
    xT_t = np.ascontiguousarray(
        xT.reshape(NK, 128, NT, 512).transpose(0, 2, 1, 3)).astype(ndt)
    cones = np.ones((CW, 2 * ST + 2), dtype=ndt)
    in_maps = []
    for c in range(N_CORES):
        h0 = c * CW
        wq = W_qkv[:, h0:h0 + CW]
        wk = W_qkv[:, C + h0:C + h0 + CW]
        wv = W_qkv[:, 2 * C + h0:2 * C + h0 + CW]
        wqkv = np.ascontiguousarray(
            np.concatenate([wq, wk, wv], axis=1)).astype(ndt)
        bq = np.stack([b_qkv[h0:h0 + CW], b_qkv[C + h0:C + h0 + CW],
                       b_qkv[2 * C + h0:2 * C + h0 + CW]],
                      axis=1).astype(np.float32)
        wo = np.ascontiguousarray(W_out[:, h0:h0 + CW]).astype(ndt)
        bo = b_out[h0:h0 + CW].reshape(CW, 1).astype(np.float32)
        in_maps.append({
            "xT": xT_t, "wqkv": wqkv, "bqkv": bq,
            "wout": wo, "bout": bo, "cones": cones,
        })
    return in_maps


def run(in_maps, trace=False):
    nc = _get_nc()
    return run_bass_kernel_spmd(nc, in_maps, core_ids=list(range(N_CORES)),
                                trace=trace)


def kernel(x, W_qkv, b_qkv, W_out, b_out):
    x = np.asarray(x, dtype=np.float32)
    W_qkv = np.asarray(W_qkv, dtype=np.float32)
    b_qkv = np.asarray(b_qkv, dtype=np.float32)
    W_out = np.asarray(W_out, dtype=np.float32)
    b_out = np.asarray(b_out, dtype=np.float32)

    in_maps = make_in_maps(x, W_qkv, b_qkv, W_out, b_out)
    res = run(in_maps, trace=False)
    # outT per core: [NT, 128, 512] tile-major -> [128 cols, 4096 tokens]
    parts = []
    for c in range(N_CORES):
        o = res.results[c]["outT"]                   # [8, 128, 512]
        parts.append(o.transpose(1, 0, 2).reshape(128, BT))
    full_outT = np.concatenate(parts, axis=0)        # [C, BT]
    return np.ascontiguousarray(full_outT.T).reshape(B, T, C)


# revision 9
# speedup vs baseline: 1.1500x; 1.0994x over previous
"""Causal self-attention (B=2, T=2048, C=1024, H=16, D=64) on 8 Trainium2 cores.

Sharding: head-parallel. Core c owns heads {2c, 2c+1} for both batches:
  - qkv projection with its 128-column slice of W_qkv (tensor parallel),
  - attention for its 4 (batch, head) pairs,
  - per-batch AllGather of per-core attention outputs Y^T (128 rows -> 1024),
  - output projection against a 128-column slice of W_out (tensor parallel).
Host assembles the 8 column slices into the full output.

On-chip layout is fully transposed (feature dims on partitions, tokens on the
free axis) so every matmul streams 512-wide moving operands:
  scoresT[s,t] = kT.T @ qT  (K=64, both heads row-tiled into the PE array)
  expT = Exp(scoresT/8)     (ACT, straight from PSUM; causal mask added to
                             PSUM as a 0/-1e9 tile before the exp)
  av    = [v | 1].T @ expT  (K=128; ones column makes row 64 the softmax
                             denominator -- no separate reduction pass)
  YT    = av[0:64] * bcast(1/av[64])  (reciprocal + K=1 ones matmul bcast)

All DRAM traffic is tile-major ([... ,128 ,512] contiguous blocks) so each
DMA is one 128-512KB descriptor run instead of 128 strided 1-2KB rows.
The AllGather is split per batch: AG(b=0) overlaps attention for b=1, and
the b=0 output projection overlaps AG(b=1).
"""
import numpy as np

import concourse.bass as bass
import concourse.mybir as mybir
import concourse.tile as tile
from concourse import bacc
from concourse.masks import make_identity
from concourse.bass_utils import run_bass_kernel_spmd

AF = mybir.ActivationFunctionType
FP32 = mybir.dt.float32

B, T, C, H, D = 2, 2048, 1024, 16, 64
N_CORES = 8
BT = B * T                      # 4096 tokens
NT = BT // 512                  # 8 token tiles of 512
NK = C // 128                   # 8 contraction tiles
HPC = H // N_CORES              # 2 heads per core
CW = HPC * D                    # 128 columns of Y / W_out per core
JT = T // 512                   # 4 token tiles per batch
ST = T // 128                   # 16 s-tiles per batch
VBLK = 2 * (D + 1)              # 130: [v_h0(64) | 1 | v_h1(64) | 1]

# matmul input dtype: "f32r" (tf32-like), "fp32", "bf16"
MM_DTYPE = "f32r"

_nc_cache = {}


def _build_nc(mm_dtype: str):
    DT = {"f32r": mybir.dt.float32r, "fp32": mybir.dt.float32,
          "bf16": mybir.dt.bfloat16}[mm_dtype]

    nc = bacc.Bacc("TRN2", target_bir_lowering=False, debug=False,
                   num_devices=N_CORES)

    # tile-major inputs: every [128, 512] block is contiguous in DRAM
    xT = nc.dram_tensor("xT", [NK, NT, 128, 512], DT,
                        kind="ExternalInput").ap()
    wqkv = nc.dram_tensor("wqkv", [C, 3 * CW], DT, kind="ExternalInput").ap()
    bqkv = nc.dram_tensor("bqkv", [CW, 3], FP32, kind="ExternalInput").ap()
    wout = nc.dram_tensor("wout", [C, CW], DT, kind="ExternalInput").ap()
    bout = nc.dram_tensor("bout", [CW, 1], FP32, kind="ExternalInput").ap()
    cones = nc.dram_tensor("cones", [CW, 2 * ST + 2], DT,
                           kind="ExternalInput").ap()
    outT = nc.dram_tensor("outT", [NT, 128, 512], FP32,
                          kind="ExternalOutput").ap()

    wqkv3 = wqkv.rearrange("(ko p) m -> p ko m", p=128)   # [128, 8, 384]
    wout3 = wout.rearrange("(ko p) m -> p ko m", p=128)   # [128, 8, 128]

    with tile.TileContext(nc) as tc:
        with tc.tile_pool(name="persist", bufs=1) as pp, \
             tc.tile_pool(name="dram", bufs=1, space="DRAM") as dram:
            qTs = pp.tile([128, BT], DT)
            kTs = pp.tile([128, BT], DT)
            vs = pp.tile([128, 2 * ST, VBLK], DT)
            YT = pp.tile([128, BT], DT)
            wq_s = pp.tile([128, NK, 3 * CW], DT)
            wo_s = pp.tile([128, NK, CW], DT)
            bq_s = pp.tile([CW, 3], FP32)
            bo_s = pp.tile([CW, 1], FP32)
            ones_s = pp.tile([1, 128], DT)
            ident = pp.tile([128, 128], FP32)
            nmask = pp.tile([128, 4, 512], FP32)

            nc.sync.dma_start(wq_s[:], wqkv3[:])
            nc.sync.dma_start(wo_s[:], wout3[:])
            nc.sync.dma_start(bq_s[:], bqkv[:])
            nc.sync.dma_start(bo_s[:], bout[:])
            nc.sync.dma_start(ones_s[:],
                              cones.rearrange("p s -> s p")[0:1, :])
            # ones columns of vs (positions 64 and 129 of each 130-block)
            nc.sync.dma_start(vs[:, :, D:D + 1],
                              cones[:, 0:2 * ST].unsqueeze(2))
            nc.sync.dma_start(vs[:, :, 2 * D + 1:2 * D + 2],
                              cones[:, 0:2 * ST].unsqueeze(2))
            make_identity(nc, ident[:])
            # nmask[dm][p,f] = 0 where f - p - 128*dm >= 0 else -1e9
            for dm in range(4):
                nc.gpsimd.memset(nmask[:, dm, :], 0.0)
                nc.gpsimd.affine_select(
                    out=nmask[:, dm, :], in_=nmask[:, dm, :],
                    compare_op=mybir.AluOpType.is_ge,
                    fill=-1e9, base=-128 * dm, channel_multiplier=-1,
                    pattern=[[1, 512]],
                )

            # per-(batch, j-tile) AllGather buffers, tile-major
            ag_in = {(b, j): dram.tile([128, 512], DT, name=f"ag_in{b}_{j}")
                     for b in range(B) for j in range(JT)}
            ag_out = {(b, j): dram.tile([N_CORES, 128, 512], DT,
                                        name=f"ag_out{b}_{j}")
                      for b in range(B) for j in range(JT)}

            # ---------------- phase 1: q/k/v projections -----------------
            with tc.tile_pool(name="ph1_sb", bufs=8) as sb1, \
                 tc.tile_pool(name="ph1_vt", bufs=3) as sbv, \
                 tc.tile_pool(name="ph1_ps", bufs=2, space="PSUM") as ps1:
                for n in range(NT):
                    ncol = slice(n * 512, (n + 1) * 512)
                    pq = ps1.tile([128, 512], FP32, tag="pq")
                    pk = ps1.tile([128, 512], FP32, tag="pk")
                    pv = ps1.tile([128, 512], FP32, tag="pv")
                    for k in range(NK):
                        xt = sb1.tile([128, 512], DT, tag="xt")
                        nc.sync.dma_start(xt[:], xT[k, n])
                        st, sp = (k == 0), (k == NK - 1)
                        nc.tensor.matmul(pq[:], wq_s[:, k, 0:128], xt[:],
                                         start=st, stop=sp)
                        nc.tensor.matmul(pk[:], wq_s[:, k, 128:256], xt[:],
                                         start=st, stop=sp)
                        nc.tensor.matmul(pv[:], wq_s[:, k, 256:384], xt[:],
                                         start=st, stop=sp)
                    nc.scalar.activation(qTs[:, ncol], pq[:], AF.Identity,
                                         bias=bq_s[:, 0:1])
                    nc.scalar.activation(kTs[:, ncol], pk[:], AF.Identity,
                                         bias=bq_s[:, 1:2])
                    vt = sbv.tile([128, 512], FP32, tag="vt")
                    nc.scalar.activation(vt[:], pv[:], AF.Identity,
                                         bias=bq_s[:, 2:3])
                    for t4 in range(4):
                        gi = n * 4 + t4
                        ptr = ps1.tile([128, 128], FP32, tag="ptr")
                        nc.tensor.transpose(ptr[:], vt[:, t4 * 128:(t4 + 1) * 128],
                                            ident[:])
                        nc.scalar.activation(vs[:, gi, 0:D], ptr[:, 0:D],
                                             AF.Copy)
                        nc.scalar.activation(vs[:, gi, D + 1:2 * D + 1],
                                             ptr[:, D:2 * D], AF.Copy)

            # -------- phase 2 (attention) + split AG + phase 3 ------------
            with tc.tile_pool(name="ph2_e", bufs=6) as sbe, \
                 tc.tile_pool(name="ph2_s", bufs=4) as sbs, \
                 tc.tile_pool(name="ph3_sb", bufs=6) as sb3, \
                 tc.tile_pool(name="ps_sc", bufs=2, space="PSUM") as ps_sc, \
                 tc.tile_pool(name="ps_av", bufs=1, space="PSUM") as ps_av, \
                 tc.tile_pool(name="ps_bc", bufs=1, space="PSUM") as ps_bc, \
                 tc.tile_pool(name="ph3_ps", bufs=1, space="PSUM") as ps3:

                def attention_batch(b):
                    for j in range(JT - 1, -1, -1):
                        col = slice(b * T + j * 512, b * T + (j + 1) * 512)
                        n_i = 4 * j + 4
                        av = [ps_av.tile([128, 512], FP32, tag=f"av{h}",
                                         name=f"av{h}_{b}_{j}")
                              for h in range(2)]
                        for i in range(n_i):
                            krange = slice(b * T + i * 128,
                                           b * T + (i + 1) * 128)
                            gi = b * ST + i
                            st, sp = (i == 0), (i == n_i - 1)
                            for h in range(2):
                                hp = slice(h * D, (h + 1) * D)
                                ps = ps_sc.tile([128, 512], FP32, tag=f"s{h}")
                                nc.tensor.matmul(
                                    ps[:], kTs[hp, krange], qTs[hp, col],
                                    start=True, stop=True,
                                    tile_position=(h * D, 0))
                                if i >= 4 * j:
                                    nc.vector.tensor_add(
                                        ps[:], ps[:], nmask[:, i - 4 * j, :])
                                e = sbe.tile([128, 512], DT, tag=f"e{h}")
                                nc.scalar.activation(e[:], ps[:], AF.Exp,
                                                     scale=0.125)
                                nc.tensor.matmul(
                                    av[h][0:D + 1, :],
                                    vs[:, gi, h * (D + 1):(h + 1) * (D + 1)],
                                    e[:], start=st, stop=sp)
                        for h in range(2):
                            # one ACT copy frees the av accumulator bank fast
                            avs = sbs.tile([D + 1, 512], FP32, tag="avs",
                                           name=f"avs_{b}_{j}_{h}")
                            nc.scalar.activation(avs[:], av[h][0:D + 1, :],
                                                 AF.Copy)
                            recip = sbs.tile([1, 512], FP32, tag="recip")
                            nc.vector.reciprocal(recip[:], avs[D:D + 1, :])
                            rr = sbs.tile([1, 512], DT, tag="rr")
                            nc.scalar.activation(rr[:], recip[:], AF.Copy)
                            pbc = ps_bc.tile([128, 512], FP32, tag="bc")
                            nc.tensor.matmul(pbc[:], ones_s[:], rr[:],
                                             start=True, stop=True)
                            nc.vector.tensor_mul(
                                YT[h * D:(h + 1) * D, col],
                                avs[0:D, :], pbc[0:D, :])
                        # ship this j-tile of YT and gather it immediately
                        nc.sync.dma_start(ag_in[b, j][:], YT[:, col])
                        nc.gpsimd.collective_compute(
                            "AllGather", mybir.AluOpType.bypass,
                            replica_groups=[list(range(N_CORES))],
                            ins=[ag_in[b, j].opt()], outs=[ag_out[b, j].opt()],
                        )

                def project_tile(b, j):
                    n = b * JT + j
                    p3 = ps3.tile([128, 512], FP32, tag="p3",
                                  name=f"p3_{b}_{j}")
                    for r in range(NK):
                        yt = sb3.tile([128, 512], DT, tag="yt")
                        nc.sync.dma_start(yt[:], ag_out[b, j][r])
                        nc.tensor.matmul(p3[:], wo_s[:, r, :], yt[:],
                                         start=(r == 0), stop=(r == NK - 1))
                    ot = sb3.tile([128, 512], FP32, tag="ot")
                    nc.scalar.activation(ot[:], p3[:], AF.Identity,
                                         bias=bo_s[:, 0:1])
                    nc.sync.dma_start(outT[n], ot[:])

                attention_batch(0)
                attention_batch(1)
                for b in range(B):
                    for j in range(JT - 1, -1, -1):
                        project_tile(b, j)

    nc.compile()
    return nc


def _get_nc():
    if MM_DTYPE not in _nc_cache:
        _nc_cache[MM_DTYPE] = _build_nc(MM_DTYPE)
    return _nc_cache[MM_DTYPE]


def _np_dt(mm_dtype: str):
    if mm_dtype == "bf16":
        import ml_dtypes
        return ml_dtypes.bfloat16
    return np.float32


def make_in_maps(x, W_qkv, b_qkv, W_out, b_out):
    ndt = _np_dt(MM_DTYPE)
    xT = x.reshape(BT, C).T                                # [C, BT]
    xT_t = np.ascontiguousarray(
        xT.reshape(NK, 128, NT, 512).transpose(0, 2, 1, 3)).astype(ndt)
    cones = np.ones((CW, 2 * ST + 2), dtype=ndt)
    in_maps = []
    for c in range(N_CORES):
        h0 = c * CW
        wq = W_qkv[:, h0:h0 + CW]
        wk = W_qkv[:, C + h0:C + h0 + CW]
        wv = W_qkv[:, 2 * C + h0:2 * C + h0 + CW]
        wqkv = np.ascontiguousarray(
            np.concatenate([wq, wk, wv], axis=1)).astype(ndt)
        bq = np.stack([b_qkv[h0:h0 + CW], b_qkv[C + h0:C + h0 + CW],
                       b_qkv[2 * C + h0:2 * C + h0 + CW]],
                      axis=1).astype(np.float32)
        wo = np.ascontiguousarray(W_out[:, h0:h0 + CW]).astype(ndt)
        bo = b_out[h0:h0 + CW].reshape(CW, 1).astype(np.float32)
        in_maps.append({
            "xT": xT_t, "wqkv": wqkv, "bqkv": bq,
            "wout": wo, "bout": bo, "cones": cones,
        })
    return in_maps


def run(in_maps, trace=False):
    nc = _get_nc()
    return run_bass_kernel_spmd(nc, in_maps, core_ids=list(range(N_CORES)),
                                trace=trace)


def kernel(x, W_qkv, b_qkv, W_out, b_out):
    x = np.asarray(x, dtype=np.float32)
    W_qkv = np.asarray(W_qkv, dtype=np.float32)
    b_qkv = np.asarray(b_qkv, dtype=np.float32)
    W_out = np.asarray(W_out, dtype=np.float32)
    b_out = np.asarray(b_out, dtype=np.float32)

    in_maps = make_in_maps(x, W_qkv, b_qkv, W_out, b_out)
    res = run(in_maps, trace=False)
    # outT per core: [NT, 128, 512] tile-major -> [128 cols, 4096 tokens]
    parts = []
    for c in range(N_CORES):
        o = res.results[c]["outT"]                   # [8, 128, 512]
        parts.append(o.transpose(1, 0, 2).reshape(128, BT))
    full_outT = np.concatenate(parts, axis=0)        # [C, BT]
    return np.ascontiguousarray(full_outT.T).reshape(B, T, C)
